# revision 1
# baseline (speedup 1.0000x reference)
"""CenterLoss (segment-reduce) kernel for Trainium2, 8 NeuronCores.

Math: out = (1/B) * sum_j sums_j / (counts_j * F)  over classes j with
counts_j > 0, where sums_j = sum_{i: label_i=j} ||feat_i - center_j||^2.

Two device algorithms (CL_ALGO):

"dot" (default): expand ||f-c||^2 = ||f||^2 - 2<f,c> + ||c||^2 and fold the
  per-class weights on the host:
      out = [ sum_i w_i * (||f_i||^2 - 2<f_i, c_{l_i}>)
              + sum_{j: count_j>0} ||c_j||^2 ] / (F * B),   w_i = 1/count_{l_i}
  counts (and so w), plus the ||c_j||^2 term, are host-side numpy from
  labels/centers. The device only produces the two per-sample scalars:
      s2_i = ||f_i||^2       (ACT square + free-dim accumulate, or DVE)
      fc_i = <f_i, c_{l_i}>  (DVE tensor_tensor_reduce)
  Features stream in as [128 part, blk, 512] tiles; the matching center row
  for every sample is fetched with the GPSIMD dma_gather ucode instruction
  (SWDGE) into the identical layout, so both reductions are straight
  elementwise+accumulate ops with no data shuffling. No segment reduce on
  device at all.

"diff": original form — per-sample d_i = ||f_i - c_{l_i}||^2 via DVE
  subtract + ACT/DVE square-accumulate, then an on-device segment reduce
  into 1024 = 32x32 class bins with a factorized one-hot (class = 32q + r)
  and one PE matmul per 128-sample block accumulating into a [32,32] PSUM
  tile. counts still come from host bincount.
"""

import os
from contextlib import ExitStack

import numpy as np

import concourse.bacc as bacc
import concourse.bass as bass
import concourse.tile as tile
from concourse import mybir
from concourse.bass_utils import run_bass_kernel_spmd

NCORES = 8
BATCH = 65536
FEAT = 512
NCLASS = 1000
SHARD = BATCH // NCORES  # 8192
P = 128
NBLK = SHARD // P  # 64
CHUNK_BLKS = int(os.environ.get("CL_CHUNK_BLKS", "8"))  # blocks per DMA chunk
NCHUNK = NBLK // CHUNK_BLKS
DMA_BUFS = int(os.environ.get("CL_DMA_BUFS", "3"))
GBUFS = int(os.environ.get("CL_GBUFS", "0")) or DMA_BUFS
QW = 32  # diff algo: class = QW*q + r; 32*32 = 1024 bins >= 1000

ALGO = os.environ.get("CL_ALGO", "diff")  # "diff" | "dot"
# Dtype knobs: "f32" or "bf16" for the streamed features / gathered centers.
FEAT_DT = os.environ.get("CL_FEAT_DT", "bf16")
CENT_DT = os.environ.get("CL_CENT_DT", "bf16")
# How many of the blocks per chunk run the square-accumulate on ACT
# (the rest run on DVE) — balances the two engines.
ACT_BLOCKS = int(os.environ.get("CL_ACT_BLOCKS", "6"))
# Batched one-hot build (broadcast APs) vs per-block tensor_scalar ops.
BATCH_ONEHOT = os.environ.get("CL_BATCH_ONEHOT", "1") == "1"
# Spread chunk gathers across SWDGE queues (0 = all on queue 0).
GQ_SPREAD = min(int(os.environ.get("CL_GQ_SPREAD", "4")), 4)
# Split each chunk's gather into N sub-gathers on distinct SWDGE queues.
GSPLIT = int(os.environ.get("CL_GSPLIT", "2"))
# Issue feature DMAs alternately from N HWDGE engines (sync, scalar).
FDMA_SPREAD = min(int(os.environ.get("CL_FDMA_SPREAD", "2")), 2)
# Split the per-chunk d-weighting mult into N pieces for finer PE overlap.
MSPLIT = int(os.environ.get("CL_MSPLIT", "1"))

TRACE = os.environ.get("CL_TRACE", "0") == "1"
# timing-only ablations for TimelineSim analysis (comma list:
# feat,gather,sub,dsq)
ABLATE = set(filter(None, os.environ.get("CL_ABLATE", "").split(",")))

_DT = {"f32": mybir.dt.float32, "bf16": mybir.dt.bfloat16}


def _np_dt(name):
    if name == "f32":
        return np.float32
    import ml_dtypes

    return ml_dtypes.bfloat16


def _bcast_ap(ap, dims):
    """Build a broadcast AP from a 2-D tile AP [P, n]: dims is a list of
    ("b", count) for broadcast (stride 0) or ("d", count) to consume the
    tile's free dim."""
    part = ap.ap[0]
    free = ap.ap[1:]
    assert len(free) == 1
    stride = free[0][0]
    out = [part]
    for kind, count in dims:
        if kind == "b":
            out.append([0, count])
        else:
            out.append([stride, count])
    return bass.AP(tensor=ap.tensor, offset=ap.offset, ap=out)


def build_module(repeat: int = 1):
    if ALGO == "dot":
        return _build_dot(repeat)
    return _build_diff(repeat)


def _build_dot(repeat: int = 1):
    """Dot-form kernel: outputs per-sample s2 and fc, [128, 2*64] packed."""
    f32 = mybir.dt.float32
    i16 = mybir.dt.int16
    fdt = _DT[FEAT_DT]
    cdt = _DT[CENT_DT]
    ddt = fdt if fdt == cdt else f32  # scratch dtype

    nc = bacc.Bacc(
        "TRN2", target_bir_lowering=False, debug=False, num_devices=NCORES,
        num_swdge_queues=max(1, GQ_SPREAD),
    )
    feat_d = nc.dram_tensor("features", [SHARD, FEAT], fdt, kind="ExternalInput")
    cent_d = nc.dram_tensor("centers", [NCLASS, FEAT], cdt, kind="ExternalInput")
    idx_d = nc.dram_tensor("labels16", [P, SHARD // 16], i16, kind="ExternalInput")
    out_d = nc.dram_tensor("out", [P, 2 * NBLK], f32, kind="ExternalOutput")

    with tile.TileContext(nc) as tc:
        with ExitStack() as ctx:
            singles = ctx.enter_context(tc.tile_pool(name="singles", bufs=1))
            fpool = ctx.enter_context(tc.tile_pool(name="fpool", bufs=DMA_BUFS))
            gpool = ctx.enter_context(tc.tile_pool(name="gpool", bufs=GBUFS))
            sqpool = ctx.enter_context(tc.tile_pool(name="sqpool", bufs=4))
            fcpool = ctx.enter_context(tc.tile_pool(name="fcpool", bufs=4))

            idx_t = singles.tile([P, SHARD // 16], i16)
            nc.sync.dma_start(out=idx_t[:], in_=idx_d.ap())

            # out columns 0:64 = s2 = ||f||^2, 64:128 = fc = <f, c_label>
            res_t = singles.tile([P, 2 * NBLK], f32)
            feat_ap = feat_d.ap().rearrange("(b p) f -> p b f", p=P)

            if repeat > 1:
                loop_cm = tc.For_i(0, repeat, 1)
                loop_cm.__enter__()

            nidx = CHUNK_BLKS * P
            for c in range(NCHUNK):
                cs = slice(c * CHUNK_BLKS, (c + 1) * CHUNK_BLKS)
                ft = fpool.tile([P, CHUNK_BLKS, FEAT], fdt)
                if "feat" not in ABLATE:
                    nc.sync.dma_start(out=ft[:], in_=feat_ap[:, cs, :])
                else:
                    nc.vector.memset(ft[:, 0, 0:8], 0)
                gt = gpool.tile([P, CHUNK_BLKS, FEAT], cdt)
                if "gather" not in ABLATE:
                    nc.gpsimd.dma_gather(
                        out_ap=gt[:],
                        in_ap=cent_d.ap(),
                        idxs_ap=idx_t[
                            :, c * (nidx // 16) : (c + 1) * (nidx // 16)
                        ],
                        num_idxs=nidx,
                        num_idxs_reg=nidx,
                        elem_size=FEAT,
                        queue_num=(c % GQ_SPREAD) if GQ_SPREAD else 0,
                    )
                else:
                    nc.vector.memset(gt[:, 0, 0:8], 0)
                for j in range(CHUNK_BLKS):
                    b = c * CHUNK_BLKS + j
                    # s2 = ||f||^2 (only needs the feature tile)
                    if "dsq" not in ABLATE:
                        sq = sqpool.tile([P, FEAT], ddt)
                        if j < ACT_BLOCKS:
                            nc.scalar.activation(
                                out=sq[:],
                                in_=ft[:, j, :],
                                func=mybir.ActivationFunctionType.Square,
                                accum_out=res_t[:, b : b + 1],
                            )
                        else:
                            nc.vector.scalar_tensor_tensor(
                                out=sq[:],
                                in0=ft[:, j, :],
                                scalar=0.0,
                                in1=ft[:, j, :],
                                op0=mybir.AluOpType.bypass,
                                op1=mybir.AluOpType.mult,
                                accum_out=res_t[:, b : b + 1],
                            )
                    # fc = <f, c_label>
                    if "sub" not in ABLATE:
                        fcs = fcpool.tile([P, FEAT], ddt)
                        nc.vector.scalar_tensor_tensor(
                            out=fcs[:],
                            in0=ft[:, j, :],
                            scalar=0.0,
                            in1=gt[:, j, :],
                            op0=mybir.AluOpType.bypass,
                            op1=mybir.AluOpType.mult,
                            accum_out=res_t[:, NBLK + b : NBLK + b + 1],
                        )
            if ABLATE:
                nc.vector.memset(res_t[:, 0:1], 0)
            nc.sync.dma_start(out=out_d.ap(), in_=res_t[:])

            if repeat > 1:
                loop_cm.__exit__(None, None, None)

    nc.compile()
    return nc


def _build_diff(repeat: int = 1):
    """Original diff-form kernel with on-device factorized segment reduce."""
    f32 = mybir.dt.float32
    i16 = mybir.dt.int16
    fdt = _DT[FEAT_DT]
    cdt = _DT[CENT_DT]
    ddt = fdt if fdt == cdt else f32  # diff/square scratch dtype
    sdt = f32  # one-hot / rhs dtype (precision: keep f32)

    nc = bacc.Bacc(
        "TRN2", target_bir_lowering=False, debug=False, num_devices=NCORES,
        num_swdge_queues=max(1, GQ_SPREAD),
    )
    feat_d = nc.dram_tensor("features", [SHARD, FEAT], fdt, kind="ExternalInput")
    cent_d = nc.dram_tensor("centers", [NCLASS, FEAT], cdt, kind="ExternalInput")
    idx_d = nc.dram_tensor("labels16", [P, SHARD // 16], i16, kind="ExternalInput")
    q_d = nc.dram_tensor("qcol", [P, NBLK], f32, kind="ExternalInput")
    r_d = nc.dram_tensor("rcol", [P, NBLK], f32, kind="ExternalInput")
    iota_d = nc.dram_tensor("iota", [P, QW], sdt, kind="ExternalInput")
    out_d = nc.dram_tensor("out", [QW, QW], f32, kind="ExternalOutput")

    with tile.TileContext(nc) as tc:
        with ExitStack() as ctx:
            singles = ctx.enter_context(tc.tile_pool(name="singles", bufs=1))
            fpool = ctx.enter_context(tc.tile_pool(name="fpool", bufs=DMA_BUFS))
            gpool = ctx.enter_context(tc.tile_pool(name="gpool", bufs=GBUFS))
            dpool = ctx.enter_context(tc.tile_pool(name="dpool", bufs=4))
            sqpool = ctx.enter_context(tc.tile_pool(name="sqpool", bufs=4))
            small = ctx.enter_context(tc.tile_pool(name="small", bufs=4))
            psum_p = ctx.enter_context(
                tc.tile_pool(name="psum", bufs=1, space="PSUM")
            )

            idx_t = singles.tile([P, SHARD // 16], i16)
            nc.sync.dma_start(out=idx_t[:], in_=idx_d.ap())
            q_t = singles.tile([P, NBLK], f32)
            nc.sync.dma_start(out=q_t[:], in_=q_d.ap())
            r_t = singles.tile([P, NBLK], f32)
            nc.sync.dma_start(out=r_t[:], in_=r_d.ap())
            iota_t = singles.tile([P, QW], sdt)
            nc.sync.dma_start(out=iota_t[:], in_=iota_d.ap())

            if BATCH_ONEHOT:
                # one-hot(q) for all blocks: [P, b, j] = (iota[j] == q[p, b])
                ohq_all = singles.tile([P, NBLK, QW], sdt)
                nc.vector.tensor_tensor(
                    out=ohq_all[:],
                    in0=_bcast_ap(iota_t[:], [("b", NBLK), ("d", QW)]),
                    in1=_bcast_ap(q_t[:], [("d", NBLK), ("b", QW)]),
                    op=mybir.AluOpType.is_equal,
                )
                ohr_all = singles.tile([P, NBLK, QW], sdt)
                nc.vector.tensor_tensor(
                    out=ohr_all[:],
                    in0=_bcast_ap(iota_t[:], [("b", NBLK), ("d", QW)]),
                    in1=_bcast_ap(r_t[:], [("d", NBLK), ("b", QW)]),
                    op=mybir.AluOpType.is_equal,
                )
                # d-weighted one-hot(r), filled per chunk
                rhs_all = singles.tile([P, NBLK, QW], sdt)

            psum_t = psum_p.tile([QW, QW], f32, space="PSUM")
            feat_ap = feat_d.ap().rearrange("(b p) f -> p b f", p=P)

            if repeat > 1:
                loop_cm = tc.For_i(0, repeat, 1)
                loop_cm.__enter__()

            nidx = CHUNK_BLKS * P  # gather indices per chunk
            for c in range(NCHUNK):
                cs = slice(c * CHUNK_BLKS, (c + 1) * CHUNK_BLKS)
                ft = fpool.tile([P, CHUNK_BLKS, FEAT], fdt)
                fengines = [nc.sync, nc.scalar, nc.vector][:FDMA_SPREAD]
                half = CHUNK_BLKS // len(fengines)
                for e, eng in enumerate(fengines):
                    eng.dma_start(
                        out=ft[:, e * half : (e + 1) * half, :],
                        in_=feat_ap[
                            :,
                            c * CHUNK_BLKS + e * half : c * CHUNK_BLKS
                            + (e + 1) * half,
                            :,
                        ],
                    )
                gt = gpool.tile([P, CHUNK_BLKS, FEAT], cdt)
                gh = CHUNK_BLKS // GSPLIT
                for g in range(GSPLIT):
                    sidx = nidx // GSPLIT
                    nc.gpsimd.dma_gather(
                        out_ap=gt[:, g * gh : (g + 1) * gh, :],
                        in_ap=cent_d.ap(),
                        idxs_ap=idx_t[
                            :,
                            c * (nidx // 16) + g * (sidx // 16) : c * (nidx // 16)
                            + (g + 1) * (sidx // 16),
                        ],
                        num_idxs=sidx,
                        num_idxs_reg=sidx,
                        elem_size=FEAT,
                        queue_num=((c * GSPLIT + g) % GQ_SPREAD)
                        if GQ_SPREAD
                        else 0,
                    )
                d_chunk = small.tile([P, CHUNK_BLKS], f32)
                for j in range(CHUNK_BLKS):
                    diff = dpool.tile([P, FEAT], ddt)
                    nc.vector.tensor_tensor(
                        out=diff[:],
                        in0=ft[:, j, :],
                        in1=gt[:, j, :],
                        op=mybir.AluOpType.subtract,
                    )
                    sq = sqpool.tile([P, FEAT], ddt)
                    if j < ACT_BLOCKS:
                        nc.scalar.activation(
                            out=sq[:],
                            in_=diff[:],
                            func=mybir.ActivationFunctionType.Square,
                            accum_out=d_chunk[:, j : j + 1],
                        )
                    else:
                        nc.vector.scalar_tensor_tensor(
                            out=sq[:],
                            in0=diff[:],
                            scalar=0.0,
                            in1=diff[:],
                            op0=mybir.AluOpType.bypass,
                            op1=mybir.AluOpType.mult,
                            accum_out=d_chunk[:, j : j + 1],
                        )
                if BATCH_ONEHOT:
                    # rhs[:, b, :] = one-hot(r)[:, b, :] * d[:, b]
                    mh = CHUNK_BLKS // MSPLIT
                    for m in range(MSPLIT):
                        ms = slice(
                            c * CHUNK_BLKS + m * mh,
                            c * CHUNK_BLKS + (m + 1) * mh,
                        )
                        nc.vector.tensor_tensor(
                            out=rhs_all[:, ms, :],
                            in0=ohr_all[:, ms, :],
                            in1=_bcast_ap(
                                d_chunk[:, m * mh : (m + 1) * mh],
                                [("d", mh), ("b", QW)],
                            ),
                            op=mybir.AluOpType.mult,
                        )
                for j in range(CHUNK_BLKS):
                    b = c * CHUNK_BLKS + j
                    if BATCH_ONEHOT:
                        lhsT = ohq_all[:, b, :]
                        rhs = rhs_all[:, b, :]
                    else:
                        ohq_tile = small.tile([P, QW], sdt, tag=f"oq{j % 4}")
                        nc.vector.tensor_scalar(
                            out=ohq_tile[:],
                            in0=iota_t[:],
                            scalar1=q_t[:, b : b + 1],
                            scalar2=None,
                            op0=mybir.AluOpType.is_equal,
                        )
                        rhs_tile = small.tile([P, QW], sdt, tag=f"rh{j % 4}")
                        nc.vector.tensor_scalar(
                            out=rhs_tile[:],
                            in0=iota_t[:],
                            scalar1=r_t[:, b : b + 1],
                            scalar2=d_chunk[:, j : j + 1],
                            op0=mybir.AluOpType.is_equal,
                            op1=mybir.AluOpType.mult,
                        )
                        lhsT = ohq_tile[:]
                        rhs = rhs_tile[:]
                    nc.tensor.matmul(
                        out=psum_t[:],
                        lhsT=lhsT,
                        rhs=rhs,
                        start=(b == 0),
                        stop=(b == NBLK - 1),
                    )
            res_t = singles.tile([QW, QW], f32)
            nc.vector.tensor_copy(out=res_t[:], in_=psum_t[:])
            nc.sync.dma_start(out=out_d.ap(), in_=res_t[:])

            if repeat > 1:
                loop_cm.__exit__(None, None, None)

    nc.compile()
    return nc


_MODULE = None


def _get_module():
    global _MODULE
    if _MODULE is None:
        _MODULE = build_module()
    return _MODULE


def make_in_maps(features, centers, labels):
    """Host-side shard + layout prep. Returns list of 8 per-core input maps."""
    fdt = _np_dt(FEAT_DT)
    cdt = _np_dt(CENT_DT)
    features = np.ascontiguousarray(np.asarray(features), dtype=np.float32)
    centers = np.ascontiguousarray(np.asarray(centers), dtype=np.float32)
    labels = np.asarray(labels).astype(np.int64, copy=False)
    if fdt is not np.float32:
        features = features.astype(fdt)
    if cdt is not np.float32:
        centers = centers.astype(cdt)

    iota = np.ascontiguousarray(
        np.broadcast_to(np.arange(QW, dtype=np.float32), (P, QW))
    )
    in_maps = []
    for c in range(NCORES):
        lab = labels[c * SHARD : (c + 1) * SHARD]
        # wrapped-16 gather index layout: idx16[i % 16, i // 16] = lab[i],
        # replicated across the 8 groups of 16 partitions.
        idx16 = np.ascontiguousarray(lab.reshape(SHARD // 16, 16).T).astype(np.int16)
        idx16 = np.ascontiguousarray(np.tile(idx16, (8, 1)))
        m = {
            "features": features[c * SHARD : (c + 1) * SHARD],
            "centers": centers,
            "labels16": idx16,
        }
        if ALGO == "diff":
            lab_blk = lab.reshape(NBLK, P).T  # [p, b] = lab[b*128+p]
            m["qcol"] = np.ascontiguousarray((lab_blk // QW).astype(np.float32))
            m["rcol"] = np.ascontiguousarray((lab_blk % QW).astype(np.float32))
            m["iota"] = iota
        in_maps.append(m)
    return in_maps


def reduce_outputs(outs, labels, centers):
    """Combine per-core device partials + host-side terms into the loss."""
    labels = np.asarray(labels).astype(np.int64, copy=False)
    counts = np.bincount(labels, minlength=NCLASS)[:NCLASS]
    if ALGO == "diff":
        tot = np.sum(np.asarray(outs, dtype=np.float64), axis=0)  # [32, 32]
        sums = tot.reshape(-1)[:NCLASS]
        per_class = np.where(
            counts > 0, sums / np.maximum(counts * FEAT, 1.0), 0.0
        )
        return np.asarray(per_class.sum() / BATCH, dtype=np.float32)

    # dot algo: outs[c] is [P, 2*NBLK] = [s2 | fc] in block layout
    w = np.zeros(NCLASS)
    w[counts > 0] = 1.0 / counts[counts > 0]
    wi = w[labels]  # [B]
    t_parts = []
    for o in outs:
        o = np.asarray(o, dtype=np.float64)
        s2 = o[:, :NBLK].T.reshape(-1)  # sample i = b*128+p  -> [SHARD]
        fc = o[:, NBLK:].T.reshape(-1)
        t_parts.append(s2 - 2.0 * fc)
    t = np.concatenate(t_parts)  # [B], sample order
    cent64 = np.asarray(centers, dtype=np.float64)
    c2 = (cent64 * cent64).sum(axis=1)  # [NCLASS]
    total = (t * wi).sum() + c2[counts > 0].sum()
    return np.asarray(total / (FEAT * BATCH), dtype=np.float32)


LAST_RESULT = None


def kernel(features, centers, labels):
    global LAST_RESULT
    nc = _get_module()
    in_maps = make_in_maps(features, centers, labels)
    res = run_bass_kernel_spmd(
        nc, in_maps, core_ids=list(range(NCORES)), trace=TRACE
    )
    LAST_RESULT = res
    outs = [r["out"] for r in res.results]
    return reduce_outputs(outs, labels, np.asarray(centers, dtype=np.float32))



# revision 2
# speedup vs baseline: 2.0735x; 2.0735x over previous
"""CenterLoss (segment-reduce) kernel for Trainium2, 8 NeuronCores.

Math: out = (1/B) * sum_j sums_j / (counts_j * F)  over classes j with
counts_j > 0, where sums_j = sum_{i: label_i=j} ||feat_i - center_j||^2.

Device algorithms (CL_ALGO):

"seg" (default): host sorts samples by label and shards the sorted order
  across cores, so each core's 8192 samples span <=128 distinct classes.
  Expanding  sums_j = A_j - 2<S_j, c_j> + count_j*||c_j||^2  with
      A_j = sum_{i in j} ||f_i||^2,   S_j = sum_{i in j} f_i,
  the device only needs two segment reductions over its own features:
    * S (per-local-class feature sums, [128, 512] f32): one PE matmul per
      128-sample block, lhsT = host-built one-hot [128 samples, 128 local
      classes] (fp8), rhs = feature block (fp8), accumulated across all 64
      blocks into a single PSUM tile.
    * s2 (per-sample ||f||^2, [128, 64] f32): ACT square+accumulate /
      DVE mult+accumulate, split across both engines.
  No centers on device, no gather: DMA traffic is 4MB features + 1MB
  one-hot per core (fp8).  All O(NCLASS*F) center math runs on the host
  in float64.

"diff": legacy fallback (handles >128 distinct classes per shard, which
  cannot happen for this problem's uniform labels): per-sample
  d_i = ||f_i - c_{l_i}||^2 via SWDGE-gathered centers, then an on-device
  factorized one-hot segment reduce into a [32,32] PSUM tile.
"""

import os
from contextlib import ExitStack

import numpy as np

import concourse.bacc as bacc
import concourse.bass as bass
import concourse.tile as tile
from concourse import mybir
from concourse.bass_utils import run_bass_kernel_spmd

NCORES = 8
BATCH = 65536
FEAT = 512
NCLASS = 1000
SHARD = BATCH // NCORES  # 8192
P = 128
NBLK = SHARD // P  # 64
KLOC = 128  # local class slots per core (seg algo)
CHUNK_BLKS = int(os.environ.get("CL_CHUNK_BLKS", "8"))  # blocks per DMA chunk
NCHUNK = NBLK // CHUNK_BLKS
DMA_BUFS = int(os.environ.get("CL_DMA_BUFS", "3"))
GBUFS = int(os.environ.get("CL_GBUFS", "0")) or DMA_BUFS
QW = 32  # diff algo: class = QW*q + r; 32*32 = 1024 bins >= 1000

ALGO = os.environ.get("CL_ALGO", "seg")  # "seg" | "diff"
# Dtype knobs: "f32" / "bf16" / "f8" for streamed features, one-hot, scratch.
FEAT_DT = os.environ.get("CL_FEAT_DT", "f8")
OH_DT = os.environ.get("CL_OH_DT", "f8")
SQ_DT = os.environ.get("CL_SQ_DT", "bf16")
CENT_DT = os.environ.get("CL_CENT_DT", "bf16")  # diff algo only
# How many of the 64 blocks run the square-accumulate on ACT (rest on DVE).
ACT_BLOCKS = int(os.environ.get("CL_ACT_BLOCKS", "36"))
# diff-algo knobs (kept for the fallback path)
BATCH_ONEHOT = os.environ.get("CL_BATCH_ONEHOT", "1") == "1"
GQ_SPREAD = min(int(os.environ.get("CL_GQ_SPREAD", "4")), 4)
GSPLIT = int(os.environ.get("CL_GSPLIT", "2"))
FDMA_SPREAD = min(int(os.environ.get("CL_FDMA_SPREAD", "2")), 2)
MSPLIT = int(os.environ.get("CL_MSPLIT", "1"))

TRACE = os.environ.get("CL_TRACE", "0") == "1"
# timing-only ablations (comma list: feat,oh,sq,mm)
ABLATE = set(filter(None, os.environ.get("CL_ABLATE", "").split(",")))

_DT = {
    "f32": mybir.dt.float32,
    "bf16": mybir.dt.bfloat16,
    "f8": mybir.dt.float8e4,
}


def _np_dt(name):
    return mybir.dt.np(_DT[name])


def _bcast_ap(ap, dims):
    """Build a broadcast AP from a 2-D tile AP [P, n]: dims is a list of
    ("b", count) for broadcast (stride 0) or ("d", count) to consume the
    tile's free dim."""
    part = ap.ap[0]
    free = ap.ap[1:]
    assert len(free) == 1
    stride = free[0][0]
    out = [part]
    for kind, count in dims:
        if kind == "b":
            out.append([0, count])
        else:
            out.append([stride, count])
    return bass.AP(tensor=ap.tensor, offset=ap.offset, ap=out)


def _act_assign(nact):
    """Spread nact ACT-square blocks evenly over the 64 blocks."""
    return [
        (b * nact) // NBLK != ((b + 1) * nact) // NBLK for b in range(NBLK)
    ]


def build_module(repeat: int = 1, algo: str | None = None):
    if (algo or ALGO) == "seg":
        return _build_seg(repeat)
    return _build_diff(repeat)


def _build_seg(repeat: int = 1):
    """Sorted-shard segment-matmul kernel: outputs S [128,512] and s2 [128,64]."""
    f32 = mybir.dt.float32
    fdt = _DT[FEAT_DT]
    odt = _DT[OH_DT]
    sqdt = _DT[SQ_DT]

    nc = bacc.Bacc(
        "TRN2", target_bir_lowering=False, debug=False, num_devices=NCORES
    )
    feat_d = nc.dram_tensor("features", [SHARD, FEAT], fdt, kind="ExternalInput")
    oh_d = nc.dram_tensor("onehot", [P, NBLK, KLOC], odt, kind="ExternalInput")
    s_d = nc.dram_tensor("S", [KLOC, FEAT], f32, kind="ExternalOutput")
    s2_d = nc.dram_tensor("s2", [P, NBLK], f32, kind="ExternalOutput")

    act_blocks = _act_assign(ACT_BLOCKS)

    with tile.TileContext(nc) as tc:
        with ExitStack() as ctx:
            singles = ctx.enter_context(tc.tile_pool(name="singles", bufs=1))
            fpool = ctx.enter_context(tc.tile_pool(name="fpool", bufs=DMA_BUFS))
            sqpool = ctx.enter_context(tc.tile_pool(name="sqpool", bufs=4))
            psum_p = ctx.enter_context(
                tc.tile_pool(name="psum", bufs=1, space="PSUM")
            )

            oh_t = singles.tile([P, NBLK, KLOC], odt)
            if "oh" not in ABLATE:
                nc.sync.dma_start(out=oh_t[:], in_=oh_d.ap())
            else:
                nc.vector.memset(oh_t[:, 0, 0:8], 0)
            s2_t = singles.tile([P, NBLK], f32)
            psum_t = psum_p.tile([KLOC, FEAT], f32, space="PSUM")
            feat_ap = feat_d.ap().rearrange("(b p) f -> p b f", p=P)

            if repeat > 1:
                loop_cm = tc.For_i(0, repeat, 1)
                loop_cm.__enter__()

            for c in range(NCHUNK):
                cs = slice(c * CHUNK_BLKS, (c + 1) * CHUNK_BLKS)
                ft = fpool.tile([P, CHUNK_BLKS, FEAT], fdt)
                if "feat" not in ABLATE:
                    eng = [nc.sync, nc.scalar][c % 2]
                    eng.dma_start(out=ft[:], in_=feat_ap[:, cs, :])
                else:
                    nc.vector.memset(ft[:, 0, 0:8], 0)
                for j in range(CHUNK_BLKS):
                    b = c * CHUNK_BLKS + j
                    if "sq" not in ABLATE:
                        sq = sqpool.tile([P, FEAT], sqdt)
                        if act_blocks[b]:
                            nc.scalar.activation(
                                out=sq[:],
                                in_=ft[:, j, :],
                                func=mybir.ActivationFunctionType.Square,
                                accum_out=s2_t[:, b : b + 1],
                            )
                        else:
                            nc.vector.scalar_tensor_tensor(
                                out=sq[:],
                                in0=ft[:, j, :],
                                scalar=0.0,
                                in1=ft[:, j, :],
                                op0=mybir.AluOpType.bypass,
                                op1=mybir.AluOpType.mult,
                                accum_out=s2_t[:, b : b + 1],
                            )
                    if "mm" not in ABLATE:
                        nc.tensor.matmul(
                            out=psum_t[:],
                            lhsT=oh_t[:, b, :],
                            rhs=ft[:, j, :],
                            start=(b == 0),
                            stop=(b == NBLK - 1),
                        )
            if ABLATE and ("sq" in ABLATE or "mm" in ABLATE):
                nc.vector.memset(s2_t[:, 0:1], 0)
            s_t = singles.tile([KLOC, FEAT], f32)
            if "mm" not in ABLATE:
                nc.vector.tensor_copy(out=s_t[:], in_=psum_t[:])
            else:
                nc.vector.memset(s_t[:, 0:8], 0)
            nc.sync.dma_start(out=s_d.ap(), in_=s_t[:])
            nc.scalar.dma_start(out=s2_d.ap(), in_=s2_t[:])

            if repeat > 1:
                loop_cm.__exit__(None, None, None)

    nc.compile()
    return nc


def _build_diff(repeat: int = 1):
    """Legacy diff-form kernel with on-device factorized segment reduce."""
    f32 = mybir.dt.float32
    i16 = mybir.dt.int16
    fdt = _DT["bf16"]
    cdt = _DT[CENT_DT]
    ddt = fdt if fdt == cdt else f32  # diff/square scratch dtype
    sdt = f32  # one-hot / rhs dtype (precision: keep f32)

    nc = bacc.Bacc(
        "TRN2", target_bir_lowering=False, debug=False, num_devices=NCORES,
        num_swdge_queues=max(1, GQ_SPREAD),
    )
    feat_d = nc.dram_tensor("features", [SHARD, FEAT], fdt, kind="ExternalInput")
    cent_d = nc.dram_tensor("centers", [NCLASS, FEAT], cdt, kind="ExternalInput")
    idx_d = nc.dram_tensor("labels16", [P, SHARD // 16], i16, kind="ExternalInput")
    q_d = nc.dram_tensor("qcol", [P, NBLK], f32, kind="ExternalInput")
    r_d = nc.dram_tensor("rcol", [P, NBLK], f32, kind="ExternalInput")
    iota_d = nc.dram_tensor("iota", [P, QW], sdt, kind="ExternalInput")
    out_d = nc.dram_tensor("out", [QW, QW], f32, kind="ExternalOutput")

    ACT_DIFF = 6  # of the 8 blocks per chunk, run this many squares on ACT

    with tile.TileContext(nc) as tc:
        with ExitStack() as ctx:
            singles = ctx.enter_context(tc.tile_pool(name="singles", bufs=1))
            fpool = ctx.enter_context(tc.tile_pool(name="fpool", bufs=DMA_BUFS))
            gpool = ctx.enter_context(tc.tile_pool(name="gpool", bufs=GBUFS))
            dpool = ctx.enter_context(tc.tile_pool(name="dpool", bufs=4))
            sqpool = ctx.enter_context(tc.tile_pool(name="sqpool", bufs=4))
            small = ctx.enter_context(tc.tile_pool(name="small", bufs=4))
            psum_p = ctx.enter_context(
                tc.tile_pool(name="psum", bufs=1, space="PSUM")
            )

            idx_t = singles.tile([P, SHARD // 16], i16)
            nc.sync.dma_start(out=idx_t[:], in_=idx_d.ap())
            q_t = singles.tile([P, NBLK], f32)
            nc.sync.dma_start(out=q_t[:], in_=q_d.ap())
            r_t = singles.tile([P, NBLK], f32)
            nc.sync.dma_start(out=r_t[:], in_=r_d.ap())
            iota_t = singles.tile([P, QW], sdt)
            nc.sync.dma_start(out=iota_t[:], in_=iota_d.ap())

            if BATCH_ONEHOT:
                ohq_all = singles.tile([P, NBLK, QW], sdt)
                nc.vector.tensor_tensor(
                    out=ohq_all[:],
                    in0=_bcast_ap(iota_t[:], [("b", NBLK), ("d", QW)]),
                    in1=_bcast_ap(q_t[:], [("d", NBLK), ("b", QW)]),
                    op=mybir.AluOpType.is_equal,
                )
                ohr_all = singles.tile([P, NBLK, QW], sdt)
                nc.vector.tensor_tensor(
                    out=ohr_all[:],
                    in0=_bcast_ap(iota_t[:], [("b", NBLK), ("d", QW)]),
                    in1=_bcast_ap(r_t[:], [("d", NBLK), ("b", QW)]),
                    op=mybir.AluOpType.is_equal,
                )
                rhs_all = singles.tile([P, NBLK, QW], sdt)

            psum_t = psum_p.tile([QW, QW], f32, space="PSUM")
            feat_ap = feat_d.ap().rearrange("(b p) f -> p b f", p=P)

            if repeat > 1:
                loop_cm = tc.For_i(0, repeat, 1)
                loop_cm.__enter__()

            nidx = CHUNK_BLKS * P  # gather indices per chunk
            for c in range(NCHUNK):
                ft = fpool.tile([P, CHUNK_BLKS, FEAT], fdt)
                fengines = [nc.sync, nc.scalar][:FDMA_SPREAD]
                half = CHUNK_BLKS // len(fengines)
                for e, eng in enumerate(fengines):
                    eng.dma_start(
                        out=ft[:, e * half : (e + 1) * half, :],
                        in_=feat_ap[
                            :,
                            c * CHUNK_BLKS + e * half : c * CHUNK_BLKS
                            + (e + 1) * half,
                            :,
                        ],
                    )
                gt = gpool.tile([P, CHUNK_BLKS, FEAT], cdt)
                gh = CHUNK_BLKS // GSPLIT
                for g in range(GSPLIT):
                    sidx = nidx // GSPLIT
                    nc.gpsimd.dma_gather(
                        out_ap=gt[:, g * gh : (g + 1) * gh, :],
                        in_ap=cent_d.ap(),
                        idxs_ap=idx_t[
                            :,
                            c * (nidx // 16) + g * (sidx // 16) : c * (nidx // 16)
                            + (g + 1) * (sidx // 16),
                        ],
                        num_idxs=sidx,
                        num_idxs_reg=sidx,
                        elem_size=FEAT,
                        queue_num=((c * GSPLIT + g) % GQ_SPREAD)
                        if GQ_SPREAD
                        else 0,
                    )
                d_chunk = small.tile([P, CHUNK_BLKS], f32)
                for j in range(CHUNK_BLKS):
                    diff = dpool.tile([P, FEAT], ddt)
                    nc.vector.tensor_tensor(
                        out=diff[:],
                        in0=ft[:, j, :],
                        in1=gt[:, j, :],
                        op=mybir.AluOpType.subtract,
                    )
                    sq = sqpool.tile([P, FEAT], ddt)
                    if j < ACT_DIFF:
                        nc.scalar.activation(
                            out=sq[:],
                            in_=diff[:],
                            func=mybir.ActivationFunctionType.Square,
                            accum_out=d_chunk[:, j : j + 1],
                        )
                    else:
                        nc.vector.scalar_tensor_tensor(
                            out=sq[:],
                            in0=diff[:],
                            scalar=0.0,
                            in1=diff[:],
                            op0=mybir.AluOpType.bypass,
                            op1=mybir.AluOpType.mult,
                            accum_out=d_chunk[:, j : j + 1],
                        )
                if BATCH_ONEHOT:
                    mh = CHUNK_BLKS // MSPLIT
                    for m in range(MSPLIT):
                        ms = slice(
                            c * CHUNK_BLKS + m * mh,
                            c * CHUNK_BLKS + (m + 1) * mh,
                        )
                        nc.vector.tensor_tensor(
                            out=rhs_all[:, ms, :],
                            in0=ohr_all[:, ms, :],
                            in1=_bcast_ap(
                                d_chunk[:, m * mh : (m + 1) * mh],
                                [("d", mh), ("b", QW)],
                            ),
                            op=mybir.AluOpType.mult,
                        )
                for j in range(CHUNK_BLKS):
                    b = c * CHUNK_BLKS + j
                    if BATCH_ONEHOT:
                        lhsT = ohq_all[:, b, :]
                        rhs = rhs_all[:, b, :]
                    else:
                        ohq_tile = small.tile([P, QW], sdt, tag=f"oq{j % 4}")
                        nc.vector.tensor_scalar(
                            out=ohq_tile[:],
                            in0=iota_t[:],
                            scalar1=q_t[:, b : b + 1],
                            scalar2=None,
                            op0=mybir.AluOpType.is_equal,
                        )
                        rhs_tile = small.tile([P, QW], sdt, tag=f"rh{j % 4}")
                        nc.vector.tensor_scalar(
                            out=rhs_tile[:],
                            in0=iota_t[:],
                            scalar1=r_t[:, b : b + 1],
                            scalar2=d_chunk[:, j : j + 1],
                            op0=mybir.AluOpType.is_equal,
                            op1=mybir.AluOpType.mult,
                        )
                        lhsT = ohq_tile[:]
                        rhs = rhs_tile[:]
                    nc.tensor.matmul(
                        out=psum_t[:],
                        lhsT=lhsT,
                        rhs=rhs,
                        start=(b == 0),
                        stop=(b == NBLK - 1),
                    )
            res_t = singles.tile([QW, QW], f32)
            nc.vector.tensor_copy(out=res_t[:], in_=psum_t[:])
            nc.sync.dma_start(out=out_d.ap(), in_=res_t[:])

            if repeat > 1:
                loop_cm.__exit__(None, None, None)

    nc.compile()
    return nc


# ---------------------------------------------------------------------------
# Host-side prep / reduce

_LAST_PREP = None  # host metadata shared between make_in_maps and reduce


def _prep(features, centers, labels):
    """Choose the algorithm for this data and build all host metadata."""
    labels = np.asarray(labels).astype(np.int64, copy=False)
    counts = np.bincount(labels, minlength=NCLASS)[:NCLASS]
    prep = {"labels": labels, "counts": counts}
    if ALGO == "seg":
        order = np.argsort(labels, kind="stable")
        slab = labels[order]
        cls_per_core = []
        ok = True
        for c in range(NCORES):
            u = np.unique(slab[c * SHARD : (c + 1) * SHARD])
            if len(u) > KLOC:
                ok = False
            cls_per_core.append(u)
        if ok:
            prep.update(
                algo="seg", order=order, slab=slab, cls_per_core=cls_per_core
            )
            return prep
    prep["algo"] = "diff"
    return prep


def make_in_maps(features, centers, labels):
    """Host-side shard + layout prep. Returns list of 8 per-core input maps."""
    global _LAST_PREP
    features = np.ascontiguousarray(np.asarray(features), dtype=np.float32)
    centers = np.ascontiguousarray(np.asarray(centers), dtype=np.float32)
    prep = _prep(features, centers, labels)
    _LAST_PREP = prep
    labels = prep["labels"]

    if prep["algo"] == "seg":
        fdt = _np_dt(FEAT_DT)
        odt = _np_dt(OH_DT)
        fq = features.astype(fdt) if fdt is not np.float32 else features
        fs = fq[prep["order"]]  # sorted by label
        in_maps = []
        for c in range(NCORES):
            sl = prep["slab"][c * SHARD : (c + 1) * SHARD]
            u = prep["cls_per_core"][c]
            lcode = np.searchsorted(u, sl)  # [SHARD] in [0, len(u))
            oh = np.zeros((SHARD, KLOC), dtype=odt)
            oh[np.arange(SHARD), lcode] = 1.0
            # sample i = b*128 + p  ->  onehot[p, b, :]
            oh = np.ascontiguousarray(
                oh.reshape(NBLK, P, KLOC).transpose(1, 0, 2)
            )
            in_maps.append(
                {
                    "features": np.ascontiguousarray(
                        fs[c * SHARD : (c + 1) * SHARD]
                    ),
                    "onehot": oh,
                }
            )
        return in_maps

    # diff fallback
    fdt = _np_dt("bf16")
    cdt = _np_dt(CENT_DT)
    feats = features.astype(fdt) if fdt is not np.float32 else features
    cents = centers.astype(cdt) if cdt is not np.float32 else centers
    iota = np.ascontiguousarray(
        np.broadcast_to(np.arange(QW, dtype=np.float32), (P, QW))
    )
    in_maps = []
    for c in range(NCORES):
        lab = labels[c * SHARD : (c + 1) * SHARD]
        idx16 = np.ascontiguousarray(lab.reshape(SHARD // 16, 16).T).astype(
            np.int16
        )
        idx16 = np.ascontiguousarray(np.tile(idx16, (8, 1)))
        lab_blk = lab.reshape(NBLK, P).T  # [p, b] = lab[b*128+p]
        in_maps.append(
            {
                "features": feats[c * SHARD : (c + 1) * SHARD],
                "centers": cents,
                "labels16": idx16,
                "qcol": np.ascontiguousarray((lab_blk // QW).astype(np.float32)),
                "rcol": np.ascontiguousarray((lab_blk % QW).astype(np.float32)),
                "iota": iota,
            }
        )
    return in_maps


def reduce_outputs(res_list, centers):
    """Combine per-core device partials + host-side terms into the loss."""
    prep = _LAST_PREP
    counts = prep["counts"]
    cent64 = np.asarray(centers, dtype=np.float64)

    if prep["algo"] == "seg":
        A = np.zeros(NCLASS)  # per-class sum ||f||^2
        T = np.zeros(NCLASS)  # per-class <S_j, c_j>
        for c in range(NCORES):
            sl = prep["slab"][c * SHARD : (c + 1) * SHARD]
            u = prep["cls_per_core"][c]
            s2 = np.asarray(res_list[c]["s2"], dtype=np.float64)
            # sample i = b*128+p  ->  s2[p, b]
            A += np.bincount(sl, weights=s2.T.reshape(-1), minlength=NCLASS)
            S = np.asarray(res_list[c]["S"], dtype=np.float64)[: len(u)]
            np.add.at(T, u, np.einsum("kf,kf->k", S, cent64[u]))
        c2 = np.einsum("jf,jf->j", cent64, cent64)
        sums = A - 2.0 * T + counts * c2
        per_class = np.where(
            counts > 0, sums / np.maximum(counts * FEAT, 1.0), 0.0
        )
        return np.asarray(per_class.sum() / BATCH, dtype=np.float32)

    # diff fallback
    tot = np.sum(
        np.asarray([r["out"] for r in res_list], dtype=np.float64), axis=0
    )
    sums = tot.reshape(-1)[:NCLASS]
    per_class = np.where(counts > 0, sums / np.maximum(counts * FEAT, 1.0), 0.0)
    return np.asarray(per_class.sum() / BATCH, dtype=np.float32)


_MODULES = {}


def _get_module(algo):
    if algo not in _MODULES:
        _MODULES[algo] = build_module(algo=algo)
    return _MODULES[algo]


LAST_RESULT = None


def kernel(features, centers, labels):
    global LAST_RESULT
    in_maps = make_in_maps(features, centers, labels)
    nc = _get_module(_LAST_PREP["algo"])
    res = run_bass_kernel_spmd(
        nc, in_maps, core_ids=list(range(NCORES)), trace=TRACE
    )
    LAST_RESULT = res
    return reduce_outputs(res.results, np.asarray(centers, dtype=np.float32))


# revision 7
# speedup vs baseline: 2.8374x; 1.3684x over previous
"""CenterLoss (segment-reduce) kernel for Trainium2, 8 NeuronCores.

Math: out = (1/B) * sum_j sums_j / (counts_j * F)  over classes j with
counts_j > 0, where sums_j = sum_{i: label_i=j} ||feat_i - center_j||^2.

Device algorithms (CL_ALGO):

"seg" (default): host sorts samples by label and shards the sorted order
  across cores, so each core's 8192 samples span <=128 distinct classes.
  The host also pre-scales each sample by sqrt(w_{label}) (w_j=1/count_j),
  so with g_i = sqrt(w)*f_i the loss folds to
      loss = [ sum_i ||g_i||^2 - 2*sum_j sqrt(w_j)<S'_j, c_j>
               + sum_{j: count_j>0} ||c_j||^2 ] / (F*B),
  where S'_j = sum_{i in j} g_i.  The device needs only:
    * S' (per-local-class scaled-feature sums, [128, 512] f32): one PE
      matmul per 128-sample block, lhsT = host-built one-hot [128 samples,
      128 local classes] (fp8), rhs = scaled feature block (fp8),
      accumulated across all 64 blocks into a single PSUM tile.
    * sum ||g||^2: ACT square+accumulate / DVE mult+accumulate over big
      multi-block slices (2 ops per chunk, split across both engines).
  No centers on device, no gather: DMA traffic is 4MB features + 1MB
  one-hot per core (fp8).  All O(NCLASS*F) center math runs on the host
  in float64.

"diff": legacy fallback (handles >128 distinct classes per shard, which
  cannot happen for this problem's uniform labels): per-sample
  d_i = ||f_i - c_{l_i}||^2 via SWDGE-gathered centers, then an on-device
  factorized one-hot segment reduce into a [32,32] PSUM tile.
"""

import os
from contextlib import ExitStack

import numpy as np

import concourse.bacc as bacc
import concourse.bass as bass
import concourse.tile as tile
from concourse import mybir
from concourse.bass_utils import run_bass_kernel_spmd

NCORES = 8
BATCH = 65536
FEAT = 512
NCLASS = 1000
SHARD = BATCH // NCORES  # 8192
P = 128
NBLK = SHARD // P  # 64
KLOC = 128  # local class slots per core (seg algo)
CHUNK_BLKS = int(os.environ.get("CL_CHUNK_BLKS", "8"))  # blocks per DMA chunk
NCHUNK = NBLK // CHUNK_BLKS
DMA_BUFS = int(os.environ.get("CL_DMA_BUFS", "3"))
GBUFS = int(os.environ.get("CL_GBUFS", "0")) or DMA_BUFS
QW = 32  # diff algo: class = QW*q + r; 32*32 = 1024 bins >= 1000

ALGO = os.environ.get("CL_ALGO", "seg")  # "seg" | "diff"
# Dtype knobs: "f32" / "bf16" / "f8" for streamed features, one-hot, scratch.
FEAT_DT = os.environ.get("CL_FEAT_DT", "f8")
OH_DT = os.environ.get("CL_OH_DT", "f8")
SQ_DT = os.environ.get("CL_SQ_DT", "bf16")
CENT_DT = os.environ.get("CL_CENT_DT", "bf16")  # diff algo only
# How many of the 64 blocks run the square-accumulate on ACT (rest on DVE).
ACT_BLOCKS = int(os.environ.get("CL_ACT_BLOCKS", "36"))
# diff-algo knobs (kept for the fallback path)
BATCH_ONEHOT = os.environ.get("CL_BATCH_ONEHOT", "1") == "1"
GQ_SPREAD = min(int(os.environ.get("CL_GQ_SPREAD", "4")), 4)
GSPLIT = int(os.environ.get("CL_GSPLIT", "2"))
FDMA_SPREAD = min(int(os.environ.get("CL_FDMA_SPREAD", "2")), 2)
MSPLIT = int(os.environ.get("CL_MSPLIT", "1"))

TRACE = os.environ.get("CL_TRACE", "0") == "1"
# timing-only ablations (comma list: feat,oh,sq,mm)
ABLATE = set(filter(None, os.environ.get("CL_ABLATE", "").split(",")))

_DT = {
    "f32": mybir.dt.float32,
    "bf16": mybir.dt.bfloat16,
    "f8": mybir.dt.float8e4,
}


def _np_dt(name):
    return mybir.dt.np(_DT[name])


def _bcast_ap(ap, dims):
    """Build a broadcast AP from a 2-D tile AP [P, n]: dims is a list of
    ("b", count) for broadcast (stride 0) or ("d", count) to consume the
    tile's free dim."""
    part = ap.ap[0]
    free = ap.ap[1:]
    assert len(free) == 1
    stride = free[0][0]
    out = [part]
    for kind, count in dims:
        if kind == "b":
            out.append([0, count])
        else:
            out.append([stride, count])
    return bass.AP(tensor=ap.tensor, offset=ap.offset, ap=out)


def _act_split(nact):
    """Per-chunk number of ACT-square blocks (of CHUNK_BLKS), evenly spread
    so that the 64-block total is nact."""
    return [
        ((c + 1) * nact) // NCHUNK - (c * nact) // NCHUNK for c in range(NCHUNK)
    ]


def build_module(repeat: int = 1, algo: str | None = None):
    if (algo or ALGO) == "seg":
        return _build_seg(repeat)
    return _build_diff(repeat)


def _build_seg(repeat: int = 1):
    """Sorted-shard segment-matmul kernel: outputs S [128,512] and s2 [128,64]."""
    f32 = mybir.dt.float32
    fdt = _DT[FEAT_DT]
    odt = _DT[OH_DT]
    sqdt = _DT[SQ_DT]

    nc = bacc.Bacc(
        "TRN2", target_bir_lowering=False, debug=False, num_devices=NCORES
    )
    feat_d = nc.dram_tensor("features", [SHARD, FEAT], fdt, kind="ExternalInput")
    oh_d = nc.dram_tensor("onehot", [P, NBLK, KLOC], odt, kind="ExternalInput")
    s_d = nc.dram_tensor("S", [KLOC, FEAT], f32, kind="ExternalOutput")
    s2_d = nc.dram_tensor("s2", [P, 2 * NCHUNK], f32, kind="ExternalOutput")

    act_split = _act_split(ACT_BLOCKS)

    with tile.TileContext(nc) as tc:
        with ExitStack() as ctx:
            singles = ctx.enter_context(tc.tile_pool(name="singles", bufs=1))
            fpool = ctx.enter_context(tc.tile_pool(name="fpool", bufs=DMA_BUFS))
            sqpool = ctx.enter_context(tc.tile_pool(name="sqpool", bufs=4))
            psum_p = ctx.enter_context(
                tc.tile_pool(name="psum", bufs=1, space="PSUM")
            )

            oh_t = singles.tile([P, NBLK, KLOC], odt)
            if "oh" not in ABLATE:
                nc.scalar.dma_start(out=oh_t[:], in_=oh_d.ap())
            else:
                nc.vector.memset(oh_t[:, 0, 0:8], 0)
            # accumulated ||g||^2 partials: col 2c = ACT, col 2c+1 = DVE
            s2_t = singles.tile([P, 2 * NCHUNK], f32)
            psum_t = psum_p.tile([KLOC, FEAT], f32, space="PSUM")
            feat_ap = feat_d.ap().rearrange("(b p) f -> p b f", p=P)

            if repeat > 1:
                loop_cm = tc.For_i(0, repeat, 1)
                loop_cm.__enter__()

            for c in range(NCHUNK):
                cs = slice(c * CHUNK_BLKS, (c + 1) * CHUNK_BLKS)
                ft = fpool.tile([P, CHUNK_BLKS, FEAT], fdt)
                if "feat" not in ABLATE:
                    nc.sync.dma_start(out=ft[:], in_=feat_ap[:, cs, :])
                else:
                    nc.vector.memset(ft[:, 0, 0:8], 0)
                if "sq" not in ABLATE:
                    a = act_split[c]
                    if a > 0:
                        sqa = sqpool.tile([P, CHUNK_BLKS, FEAT], sqdt, tag="sqa")
                        nc.scalar.activation(
                            out=sqa[:, :a, :],
                            in_=ft[:, :a, :],
                            func=mybir.ActivationFunctionType.Square,
                            accum_out=s2_t[:, 2 * c : 2 * c + 1],
                        )
                    if a < CHUNK_BLKS:
                        sqd = sqpool.tile([P, CHUNK_BLKS, FEAT], sqdt, tag="sqd")
                        nc.vector.scalar_tensor_tensor(
                            out=sqd[:, a:, :],
                            in0=ft[:, a:, :],
                            scalar=0.0,
                            in1=ft[:, a:, :],
                            op0=mybir.AluOpType.bypass,
                            op1=mybir.AluOpType.mult,
                            accum_out=s2_t[:, 2 * c + 1 : 2 * c + 2],
                        )
                if "mm" not in ABLATE:
                    for j in range(CHUNK_BLKS):
                        b = c * CHUNK_BLKS + j
                        nc.tensor.matmul(
                            out=psum_t[:],
                            lhsT=oh_t[:, b, :],
                            rhs=ft[:, j, :],
                            start=(b == 0),
                            stop=(b == NBLK - 1),
                        )
            if ABLATE and "sq" in ABLATE:
                nc.vector.memset(s2_t[:, 0:1], 0)
            s_t = singles.tile([KLOC, FEAT], f32)
            if "mm" not in ABLATE:
                nc.vector.tensor_copy(out=s_t[:], in_=psum_t[:])
            else:
                nc.vector.memset(s_t[:, 0:8], 0)
            nc.sync.dma_start(out=s_d.ap(), in_=s_t[:])
            nc.scalar.dma_start(out=s2_d.ap(), in_=s2_t[:])

            if repeat > 1:
                loop_cm.__exit__(None, None, None)

    nc.compile()
    return nc


def _build_diff(repeat: int = 1):
    """Legacy diff-form kernel with on-device factorized segment reduce."""
    f32 = mybir.dt.float32
    i16 = mybir.dt.int16
    fdt = _DT["bf16"]
    cdt = _DT[CENT_DT]
    ddt = fdt if fdt == cdt else f32  # diff/square scratch dtype
    sdt = f32  # one-hot / rhs dtype (precision: keep f32)

    nc = bacc.Bacc(
        "TRN2", target_bir_lowering=False, debug=False, num_devices=NCORES,
        num_swdge_queues=max(1, GQ_SPREAD),
    )
    feat_d = nc.dram_tensor("features", [SHARD, FEAT], fdt, kind="ExternalInput")
    cent_d = nc.dram_tensor("centers", [NCLASS, FEAT], cdt, kind="ExternalInput")
    idx_d = nc.dram_tensor("labels16", [P, SHARD // 16], i16, kind="ExternalInput")
    q_d = nc.dram_tensor("qcol", [P, NBLK], f32, kind="ExternalInput")
    r_d = nc.dram_tensor("rcol", [P, NBLK], f32, kind="ExternalInput")
    iota_d = nc.dram_tensor("iota", [P, QW], sdt, kind="ExternalInput")
    out_d = nc.dram_tensor("out", [QW, QW], f32, kind="ExternalOutput")

    ACT_DIFF = 6  # of the 8 blocks per chunk, run this many squares on ACT

    with tile.TileContext(nc) as tc:
        with ExitStack() as ctx:
            singles = ctx.enter_context(tc.tile_pool(name="singles", bufs=1))
            fpool = ctx.enter_context(tc.tile_pool(name="fpool", bufs=DMA_BUFS))
            gpool = ctx.enter_context(tc.tile_pool(name="gpool", bufs=GBUFS))
            dpool = ctx.enter_context(tc.tile_pool(name="dpool", bufs=4))
            sqpool = ctx.enter_context(tc.tile_pool(name="sqpool", bufs=4))
            small = ctx.enter_context(tc.tile_pool(name="small", bufs=4))
            psum_p = ctx.enter_context(
                tc.tile_pool(name="psum", bufs=1, space="PSUM")
            )

            idx_t = singles.tile([P, SHARD // 16], i16)
            nc.sync.dma_start(out=idx_t[:], in_=idx_d.ap())
            q_t = singles.tile([P, NBLK], f32)
            nc.sync.dma_start(out=q_t[:], in_=q_d.ap())
            r_t = singles.tile([P, NBLK], f32)
            nc.sync.dma_start(out=r_t[:], in_=r_d.ap())
            iota_t = singles.tile([P, QW], sdt)
            nc.sync.dma_start(out=iota_t[:], in_=iota_d.ap())

            if BATCH_ONEHOT:
                ohq_all = singles.tile([P, NBLK, QW], sdt)
                nc.vector.tensor_tensor(
                    out=ohq_all[:],
                    in0=_bcast_ap(iota_t[:], [("b", NBLK), ("d", QW)]),
                    in1=_bcast_ap(q_t[:], [("d", NBLK), ("b", QW)]),
                    op=mybir.AluOpType.is_equal,
                )
                ohr_all = singles.tile([P, NBLK, QW], sdt)
                nc.vector.tensor_tensor(
                    out=ohr_all[:],
                    in0=_bcast_ap(iota_t[:], [("b", NBLK), ("d", QW)]),
                    in1=_bcast_ap(r_t[:], [("d", NBLK), ("b", QW)]),
                    op=mybir.AluOpType.is_equal,
                )
                rhs_all = singles.tile([P, NBLK, QW], sdt)

            psum_t = psum_p.tile([QW, QW], f32, space="PSUM")
            feat_ap = feat_d.ap().rearrange("(b p) f -> p b f", p=P)

            if repeat > 1:
                loop_cm = tc.For_i(0, repeat, 1)
                loop_cm.__enter__()

            nidx = CHUNK_BLKS * P  # gather indices per chunk
            for c in range(NCHUNK):
                ft = fpool.tile([P, CHUNK_BLKS, FEAT], fdt)
                fengines = [nc.sync, nc.scalar][:FDMA_SPREAD]
                half = CHUNK_BLKS // len(fengines)
                for e, eng in enumerate(fengines):
                    eng.dma_start(
                        out=ft[:, e * half : (e + 1) * half, :],
                        in_=feat_ap[
                            :,
                            c * CHUNK_BLKS + e * half : c * CHUNK_BLKS
                            + (e + 1) * half,
                            :,
                        ],
                    )
                gt = gpool.tile([P, CHUNK_BLKS, FEAT], cdt)
                gh = CHUNK_BLKS // GSPLIT
                for g in range(GSPLIT):
                    sidx = nidx // GSPLIT
                    nc.gpsimd.dma_gather(
                        out_ap=gt[:, g * gh : (g + 1) * gh, :],
                        in_ap=cent_d.ap(),
                        idxs_ap=idx_t[
                            :,
                            c * (nidx // 16) + g * (sidx // 16) : c * (nidx // 16)
                            + (g + 1) * (sidx // 16),
                        ],
                        num_idxs=sidx,
                        num_idxs_reg=sidx,
                        elem_size=FEAT,
                        queue_num=((c * GSPLIT + g) % GQ_SPREAD)
                        if GQ_SPREAD
                        else 0,
                    )
                d_chunk = small.tile([P, CHUNK_BLKS], f32)
                for j in range(CHUNK_BLKS):
                    diff = dpool.tile([P, FEAT], ddt)
                    nc.vector.tensor_tensor(
                        out=diff[:],
                        in0=ft[:, j, :],
                        in1=gt[:, j, :],
                        op=mybir.AluOpType.subtract,
                    )
                    sq = sqpool.tile([P, FEAT], ddt)
                    if j < ACT_DIFF:
                        nc.scalar.activation(
                            out=sq[:],
                            in_=diff[:],
                            func=mybir.ActivationFunctionType.Square,
                            accum_out=d_chunk[:, j : j + 1],
                        )
                    else:
                        nc.vector.scalar_tensor_tensor(
                            out=sq[:],
                            in0=diff[:],
                            scalar=0.0,
                            in1=diff[:],
                            op0=mybir.AluOpType.bypass,
                            op1=mybir.AluOpType.mult,
                            accum_out=d_chunk[:, j : j + 1],
                        )
                if BATCH_ONEHOT:
                    mh = CHUNK_BLKS // MSPLIT
                    for m in range(MSPLIT):
                        ms = slice(
                            c * CHUNK_BLKS + m * mh,
                            c * CHUNK_BLKS + (m + 1) * mh,
                        )
                        nc.vector.tensor_tensor(
                            out=rhs_all[:, ms, :],
                            in0=ohr_all[:, ms, :],
                            in1=_bcast_ap(
                                d_chunk[:, m * mh : (m + 1) * mh],
                                [("d", mh), ("b", QW)],
                            ),
                            op=mybir.AluOpType.mult,
                        )
                for j in range(CHUNK_BLKS):
                    b = c * CHUNK_BLKS + j
                    if BATCH_ONEHOT:
                        lhsT = ohq_all[:, b, :]
                        rhs = rhs_all[:, b, :]
                    else:
                        ohq_tile = small.tile([P, QW], sdt, tag=f"oq{j % 4}")
                        nc.vector.tensor_scalar(
                            out=ohq_tile[:],
                            in0=iota_t[:],
                            scalar1=q_t[:, b : b + 1],
                            scalar2=None,
                            op0=mybir.AluOpType.is_equal,
                        )
                        rhs_tile = small.tile([P, QW], sdt, tag=f"rh{j % 4}")
                        nc.vector.tensor_scalar(
                            out=rhs_tile[:],
                            in0=iota_t[:],
                            scalar1=r_t[:, b : b + 1],
                            scalar2=d_chunk[:, j : j + 1],
                            op0=mybir.AluOpType.is_equal,
                            op1=mybir.AluOpType.mult,
                        )
                        lhsT = ohq_tile[:]
                        rhs = rhs_tile[:]
                    nc.tensor.matmul(
                        out=psum_t[:],
                        lhsT=lhsT,
                        rhs=rhs,
                        start=(b == 0),
                        stop=(b == NBLK - 1),
                    )
            res_t = singles.tile([QW, QW], f32)
            nc.vector.tensor_copy(out=res_t[:], in_=psum_t[:])
            nc.sync.dma_start(out=out_d.ap(), in_=res_t[:])

            if repeat > 1:
                loop_cm.__exit__(None, None, None)

    nc.compile()
    return nc


# ---------------------------------------------------------------------------
# Host-side prep / reduce

_LAST_PREP = None  # host metadata shared between make_in_maps and reduce


def _prep(features, centers, labels):
    """Choose the algorithm for this data and build all host metadata."""
    labels = np.asarray(labels).astype(np.int64, copy=False)
    counts = np.bincount(labels, minlength=NCLASS)[:NCLASS]
    prep = {"labels": labels, "counts": counts}
    if ALGO == "seg":
        order = np.argsort(labels, kind="stable")
        slab = labels[order]
        cls_per_core = []
        ok = True
        for c in range(NCORES):
            u = np.unique(slab[c * SHARD : (c + 1) * SHARD])
            if len(u) > KLOC:
                ok = False
            cls_per_core.append(u)
        if ok:
            prep.update(
                algo="seg", order=order, slab=slab, cls_per_core=cls_per_core
            )
            return prep
    prep["algo"] = "diff"
    return prep


def make_in_maps(features, centers, labels):
    """Host-side shard + layout prep. Returns list of 8 per-core input maps."""
    global _LAST_PREP
    features = np.ascontiguousarray(np.asarray(features), dtype=np.float32)
    centers = np.ascontiguousarray(np.asarray(centers), dtype=np.float32)
    prep = _prep(features, centers, labels)
    _LAST_PREP = prep
    labels = prep["labels"]

    if prep["algo"] == "seg":
        fdt = _np_dt(FEAT_DT)
        odt = _np_dt(OH_DT)
        counts = prep["counts"]
        w = np.zeros(NCLASS, np.float32)
        nz = counts > 0
        w[nz] = 1.0 / counts[nz]
        scaled = features * np.sqrt(w)[labels][:, None]
        fq = scaled.astype(fdt) if fdt is not np.float32 else scaled
        fs = fq[prep["order"]]  # sorted by label
        in_maps = []
        for c in range(NCORES):
            sl = prep["slab"][c * SHARD : (c + 1) * SHARD]
            u = prep["cls_per_core"][c]
            lcode = np.searchsorted(u, sl)  # [SHARD] in [0, len(u))
            oh = np.zeros((SHARD, KLOC), dtype=odt)
            oh[np.arange(SHARD), lcode] = 1.0
            # sample i = b*128 + p  ->  onehot[p, b, :]
            oh = np.ascontiguousarray(
                oh.reshape(NBLK, P, KLOC).transpose(1, 0, 2)
            )
            in_maps.append(
                {
                    "features": np.ascontiguousarray(
                        fs[c * SHARD : (c + 1) * SHARD]
                    ),
                    "onehot": oh,
                }
            )
        return in_maps

    # diff fallback
    fdt = _np_dt("bf16")
    cdt = _np_dt(CENT_DT)
    feats = features.astype(fdt) if fdt is not np.float32 else features
    cents = centers.astype(cdt) if cdt is not np.float32 else centers
    iota = np.ascontiguousarray(
        np.broadcast_to(np.arange(QW, dtype=np.float32), (P, QW))
    )
    in_maps = []
    for c in range(NCORES):
        lab = labels[c * SHARD : (c + 1) * SHARD]
        idx16 = np.ascontiguousarray(lab.reshape(SHARD // 16, 16).T).astype(
            np.int16
        )
        idx16 = np.ascontiguousarray(np.tile(idx16, (8, 1)))
        lab_blk = lab.reshape(NBLK, P).T  # [p, b] = lab[b*128+p]
        in_maps.append(
            {
                "features": feats[c * SHARD : (c + 1) * SHARD],
                "centers": cents,
                "labels16": idx16,
                "qcol": np.ascontiguousarray((lab_blk // QW).astype(np.float32)),
                "rcol": np.ascontiguousarray((lab_blk % QW).astype(np.float32)),
                "iota": iota,
            }
        )
    return in_maps


def reduce_outputs(res_list, centers):
    """Combine per-core device partials + host-side terms into the loss."""
    prep = _LAST_PREP
    counts = prep["counts"]
    cent64 = np.asarray(centers, dtype=np.float64)

    if prep["algo"] == "seg":
        sqrt_w = np.zeros(NCLASS)
        nz = counts > 0
        sqrt_w[nz] = np.sqrt(1.0 / counts[nz])
        a_total = 0.0  # sum_i ||g_i||^2 = sum_j w_j A_j
        t_total = 0.0  # sum_j sqrt(w_j) <S'_j, c_j> = sum_j w_j <S_j, c_j>
        for c in range(NCORES):
            u = prep["cls_per_core"][c]
            a_total += float(
                np.asarray(res_list[c]["s2"], dtype=np.float64).sum()
            )
            S = np.asarray(res_list[c]["S"], dtype=np.float64)[: len(u)]
            t_total += float(
                np.einsum("kf,kf->k", S, cent64[u]).dot(sqrt_w[u])
            )
        c2 = float(np.einsum("jf,jf->", cent64[nz], cent64[nz]))
        total = (a_total - 2.0 * t_total + c2) / (FEAT * BATCH)
        return np.asarray(total, dtype=np.float32)

    # diff fallback
    tot = np.sum(
        np.asarray([r["out"] for r in res_list], dtype=np.float64), axis=0
    )
    sums = tot.reshape(-1)[:NCLASS]
    per_class = np.where(counts > 0, sums / np.maximum(counts * FEAT, 1.0), 0.0)
    return np.asarray(per_class.sum() / BATCH, dtype=np.float32)


_MODULES = {}


def _get_module(algo):
    if algo not in _MODULES:
        _MODULES[algo] = build_module(algo=algo)
    return _MODULES[algo]


LAST_RESULT = None


def kernel(features, centers, labels):
    global LAST_RESULT
    in_maps = make_in_maps(features, centers, labels)
    nc = _get_module(_LAST_PREP["algo"])
    res = run_bass_kernel_spmd(
        nc, in_maps, core_ids=list(range(NCORES)), trace=TRACE
    )
    LAST_RESULT = res
    return reduce_outputs(res.results, np.asarray(centers, dtype=np.float32))


# revision 11
# speedup vs baseline: 3.5188x; 1.2401x over previous
"""CenterLoss (segment-reduce) kernel for Trainium2, 8 NeuronCores.

Math: out = (1/B) * sum_j sums_j / (counts_j * F)  over classes j with
counts_j > 0, where sums_j = sum_{i: label_i=j} ||feat_i - center_j||^2.

Device algorithms (CL_ALGO):

"seg" (default): host sorts samples by label and shards the sorted order
  across cores, so each core's 8192 samples span <=128 distinct classes.
  The host also pre-scales each sample by sqrt(w_{label}) (w_j=1/count_j),
  so with g_i = sqrt(w)*f_i the loss folds to
      loss = [ sum_i ||g_i||^2 - 2*sum_j sqrt(w_j)<S'_j, c_j>
               + sum_{j: count_j>0} ||c_j||^2 ] / (F*B),
  where S'_j = sum_{i in j} g_i.  The device needs only:
    * S' (per-local-class scaled-feature sums, [128, 512] f32): one PE
      matmul per 128-sample block, lhsT = host-built one-hot [128 samples,
      128 local classes] (fp8), rhs = scaled feature block (fp8),
      accumulated across all 64 blocks into a single PSUM tile.
    * sum ||g||^2: ACT square+accumulate / DVE mult+accumulate over big
      multi-block slices (2 ops per chunk, split across both engines).
  No centers on device, no gather: DMA traffic is 4MB features + 1MB
  one-hot per core (fp8).  All O(NCLASS*F) center math runs on the host
  in float64.

"diff": legacy fallback (handles >128 distinct classes per shard, which
  cannot happen for this problem's uniform labels): per-sample
  d_i = ||f_i - c_{l_i}||^2 via SWDGE-gathered centers, then an on-device
  factorized one-hot segment reduce into a [32,32] PSUM tile.
"""

import os
from contextlib import ExitStack

import numpy as np

import concourse.bacc as bacc
import concourse.bass as bass
import concourse.tile as tile
from concourse import mybir
from concourse.bass_utils import run_bass_kernel_spmd

NCORES = 8
BATCH = 65536
FEAT = 512
NCLASS = 1000
SHARD = BATCH // NCORES  # 8192
P = 128
NBLK = SHARD // P  # 64
KLOC = 128  # local class slots per core (seg algo)
CHUNK_BLKS = int(os.environ.get("CL_CHUNK_BLKS", "8"))  # blocks per DMA chunk
NCHUNK = NBLK // CHUNK_BLKS
DMA_BUFS = int(os.environ.get("CL_DMA_BUFS", "8"))
FSPLIT = os.environ.get("CL_FSPLIT", "1") == "1"  # split chunk DMA at ACT/DVE cut
PSUM_DMA = os.environ.get("CL_PSUM_DMA", "0") == "1"  # DMA straight from PSUM (unsupported)
GBUFS = int(os.environ.get("CL_GBUFS", "0")) or DMA_BUFS
QW = 32  # diff algo: class = QW*q + r; 32*32 = 1024 bins >= 1000

ALGO = os.environ.get("CL_ALGO", "seg")  # "seg" | "diff"
# Dtype knobs: "f32" / "bf16" / "f8" for streamed features, one-hot, scratch.
FEAT_DT = os.environ.get("CL_FEAT_DT", "f8")
OH_DT = os.environ.get("CL_OH_DT", "f8")
SQ_DT = os.environ.get("CL_SQ_DT", "bf16")
CENT_DT = os.environ.get("CL_CENT_DT", "bf16")  # diff algo only
# How many of the 64 blocks run the square-accumulate on ACT (rest on DVE).
ACT_BLOCKS = int(os.environ.get("CL_ACT_BLOCKS", "36"))
# diff-algo knobs (kept for the fallback path)
BATCH_ONEHOT = os.environ.get("CL_BATCH_ONEHOT", "1") == "1"
GQ_SPREAD = min(int(os.environ.get("CL_GQ_SPREAD", "4")), 4)
GSPLIT = int(os.environ.get("CL_GSPLIT", "2"))
FDMA_SPREAD = min(int(os.environ.get("CL_FDMA_SPREAD", "2")), 2)
MSPLIT = int(os.environ.get("CL_MSPLIT", "1"))

TRACE = os.environ.get("CL_TRACE", "0") == "1"
# timing-only ablations (comma list: feat,oh,sq,mm)
ABLATE = set(filter(None, os.environ.get("CL_ABLATE", "").split(",")))

_DT = {
    "f32": mybir.dt.float32,
    "bf16": mybir.dt.bfloat16,
    "f8": mybir.dt.float8e4,
}


def _np_dt(name):
    return mybir.dt.np(_DT[name])


def _bcast_ap(ap, dims):
    """Build a broadcast AP from a 2-D tile AP [P, n]: dims is a list of
    ("b", count) for broadcast (stride 0) or ("d", count) to consume the
    tile's free dim."""
    part = ap.ap[0]
    free = ap.ap[1:]
    assert len(free) == 1
    stride = free[0][0]
    out = [part]
    for kind, count in dims:
        if kind == "b":
            out.append([0, count])
        else:
            out.append([stride, count])
    return bass.AP(tensor=ap.tensor, offset=ap.offset, ap=out)


def _act_split(nact):
    """Per-chunk number of ACT-square blocks (of CHUNK_BLKS), evenly spread
    so that the 64-block total is nact."""
    return [
        ((c + 1) * nact) // NCHUNK - (c * nact) // NCHUNK for c in range(NCHUNK)
    ]


def build_module(repeat: int = 1, algo: str | None = None):
    if (algo or ALGO) == "seg":
        return _build_seg(repeat)
    return _build_diff(repeat)


def _build_seg(repeat: int = 1):
    """Sorted-shard segment-matmul kernel: outputs S [128,512] and s2 [128,64]."""
    f32 = mybir.dt.float32
    fdt = _DT[FEAT_DT]
    odt = _DT[OH_DT]
    sqdt = _DT[SQ_DT]

    nc = bacc.Bacc(
        "TRN2", target_bir_lowering=False, debug=False, num_devices=NCORES
    )
    feat_d = nc.dram_tensor("features", [SHARD, FEAT], fdt, kind="ExternalInput")
    oh_d = nc.dram_tensor("onehot", [P, NBLK, KLOC], odt, kind="ExternalInput")
    s_d = nc.dram_tensor("S", [KLOC, FEAT], f32, kind="ExternalOutput")
    s2_d = nc.dram_tensor("s2", [P, 2 * NCHUNK], f32, kind="ExternalOutput")

    act_split = _act_split(ACT_BLOCKS)

    with tile.TileContext(nc) as tc:
        with ExitStack() as ctx:
            singles = ctx.enter_context(tc.tile_pool(name="singles", bufs=1))
            fpool = ctx.enter_context(tc.tile_pool(name="fpool", bufs=DMA_BUFS))
            sqpool = ctx.enter_context(tc.tile_pool(name="sqpool", bufs=4))
            psum_p = ctx.enter_context(
                tc.tile_pool(name="psum", bufs=1, space="PSUM")
            )

            oh_t = singles.tile([P, NBLK, KLOC], odt)
            if "oh" not in ABLATE:
                nc.scalar.dma_start(out=oh_t[:], in_=oh_d.ap())
            else:
                nc.vector.memset(oh_t[:, 0, 0:8], 0)
            # accumulated ||g||^2 partials: col 2c = ACT, col 2c+1 = DVE
            s2_t = singles.tile([P, 2 * NCHUNK], f32)
            psum_t = psum_p.tile([KLOC, FEAT], f32, space="PSUM")
            feat_ap = feat_d.ap().rearrange("(b p) f -> p b f", p=P)

            if repeat > 1:
                loop_cm = tc.For_i(0, repeat, 1)
                loop_cm.__enter__()

            for c in range(NCHUNK):
                cs = slice(c * CHUNK_BLKS, (c + 1) * CHUNK_BLKS)
                ft = fpool.tile([P, CHUNK_BLKS, FEAT], fdt)
                a = act_split[c]
                if "feat" not in ABLATE:
                    if FSPLIT and 0 < a < CHUNK_BLKS:
                        c0 = c * CHUNK_BLKS
                        nc.sync.dma_start(
                            out=ft[:, :a, :], in_=feat_ap[:, c0 : c0 + a, :]
                        )
                        nc.sync.dma_start(
                            out=ft[:, a:, :],
                            in_=feat_ap[:, c0 + a : c0 + CHUNK_BLKS, :],
                        )
                    else:
                        nc.sync.dma_start(out=ft[:], in_=feat_ap[:, cs, :])
                else:
                    nc.vector.memset(ft[:, 0, 0:8], 0)
                if "sq" not in ABLATE:
                    if a > 0:
                        sqa = sqpool.tile([P, CHUNK_BLKS, FEAT], sqdt, tag="sqa")
                        nc.scalar.activation(
                            out=sqa[:, :a, :],
                            in_=ft[:, :a, :],
                            func=mybir.ActivationFunctionType.Square,
                            accum_out=s2_t[:, 2 * c : 2 * c + 1],
                        )
                    if a < CHUNK_BLKS:
                        sqd = sqpool.tile([P, CHUNK_BLKS, FEAT], sqdt, tag="sqd")
                        nc.vector.scalar_tensor_tensor(
                            out=sqd[:, a:, :],
                            in0=ft[:, a:, :],
                            scalar=0.0,
                            in1=ft[:, a:, :],
                            op0=mybir.AluOpType.bypass,
                            op1=mybir.AluOpType.mult,
                            accum_out=s2_t[:, 2 * c + 1 : 2 * c + 2],
                        )
                if "mm" not in ABLATE:
                    for j in range(CHUNK_BLKS):
                        b = c * CHUNK_BLKS + j
                        nc.tensor.matmul(
                            out=psum_t[:],
                            lhsT=oh_t[:, b, :],
                            rhs=ft[:, j, :],
                            start=(b == 0),
                            stop=(b == NBLK - 1),
                        )
            if ABLATE and "sq" in ABLATE:
                nc.vector.memset(s2_t[:, 0:1], 0)
            if "mm" not in ABLATE and PSUM_DMA:
                nc.sync.dma_start(out=s_d.ap(), in_=psum_t[:])
            else:
                s_t = singles.tile([KLOC, FEAT], f32)
                if "mm" not in ABLATE:
                    nc.vector.tensor_copy(out=s_t[:], in_=psum_t[:])
                else:
                    nc.vector.memset(s_t[:, 0:8], 0)
                nc.sync.dma_start(out=s_d.ap(), in_=s_t[:])
            nc.scalar.dma_start(out=s2_d.ap(), in_=s2_t[:])

            if repeat > 1:
                loop_cm.__exit__(None, None, None)

    nc.compile()
    return nc


def _build_diff(repeat: int = 1):
    """Legacy diff-form kernel with on-device factorized segment reduce."""
    f32 = mybir.dt.float32
    i16 = mybir.dt.int16
    fdt = _DT["bf16"]
    cdt = _DT[CENT_DT]
    ddt = fdt if fdt == cdt else f32  # diff/square scratch dtype
    sdt = f32  # one-hot / rhs dtype (precision: keep f32)

    nc = bacc.Bacc(
        "TRN2", target_bir_lowering=False, debug=False, num_devices=NCORES,
        num_swdge_queues=max(1, GQ_SPREAD),
    )
    feat_d = nc.dram_tensor("features", [SHARD, FEAT], fdt, kind="ExternalInput")
    cent_d = nc.dram_tensor("centers", [NCLASS, FEAT], cdt, kind="ExternalInput")
    idx_d = nc.dram_tensor("labels16", [P, SHARD // 16], i16, kind="ExternalInput")
    q_d = nc.dram_tensor("qcol", [P, NBLK], f32, kind="ExternalInput")
    r_d = nc.dram_tensor("rcol", [P, NBLK], f32, kind="ExternalInput")
    iota_d = nc.dram_tensor("iota", [P, QW], sdt, kind="ExternalInput")
    out_d = nc.dram_tensor("out", [QW, QW], f32, kind="ExternalOutput")

    ACT_DIFF = 6  # of the 8 blocks per chunk, run this many squares on ACT

    with tile.TileContext(nc) as tc:
        with ExitStack() as ctx:
            singles = ctx.enter_context(tc.tile_pool(name="singles", bufs=1))
            fpool = ctx.enter_context(tc.tile_pool(name="fpool", bufs=DMA_BUFS))
            gpool = ctx.enter_context(tc.tile_pool(name="gpool", bufs=GBUFS))
            dpool = ctx.enter_context(tc.tile_pool(name="dpool", bufs=4))
            sqpool = ctx.enter_context(tc.tile_pool(name="sqpool", bufs=4))
            small = ctx.enter_context(tc.tile_pool(name="small", bufs=4))
            psum_p = ctx.enter_context(
                tc.tile_pool(name="psum", bufs=1, space="PSUM")
            )

            idx_t = singles.tile([P, SHARD // 16], i16)
            nc.sync.dma_start(out=idx_t[:], in_=idx_d.ap())
            q_t = singles.tile([P, NBLK], f32)
            nc.sync.dma_start(out=q_t[:], in_=q_d.ap())
            r_t = singles.tile([P, NBLK], f32)
            nc.sync.dma_start(out=r_t[:], in_=r_d.ap())
            iota_t = singles.tile([P, QW], sdt)
            nc.sync.dma_start(out=iota_t[:], in_=iota_d.ap())

            if BATCH_ONEHOT:
                ohq_all = singles.tile([P, NBLK, QW], sdt)
                nc.vector.tensor_tensor(
                    out=ohq_all[:],
                    in0=_bcast_ap(iota_t[:], [("b", NBLK), ("d", QW)]),
                    in1=_bcast_ap(q_t[:], [("d", NBLK), ("b", QW)]),
                    op=mybir.AluOpType.is_equal,
                )
                ohr_all = singles.tile([P, NBLK, QW], sdt)
                nc.vector.tensor_tensor(
                    out=ohr_all[:],
                    in0=_bcast_ap(iota_t[:], [("b", NBLK), ("d", QW)]),
                    in1=_bcast_ap(r_t[:], [("d", NBLK), ("b", QW)]),
                    op=mybir.AluOpType.is_equal,
                )
                rhs_all = singles.tile([P, NBLK, QW], sdt)

            psum_t = psum_p.tile([QW, QW], f32, space="PSUM")
            feat_ap = feat_d.ap().rearrange("(b p) f -> p b f", p=P)

            if repeat > 1:
                loop_cm = tc.For_i(0, repeat, 1)
                loop_cm.__enter__()

            nidx = CHUNK_BLKS * P  # gather indices per chunk
            for c in range(NCHUNK):
                ft = fpool.tile([P, CHUNK_BLKS, FEAT], fdt)
                fengines = [nc.sync, nc.scalar][:FDMA_SPREAD]
                half = CHUNK_BLKS // len(fengines)
                for e, eng in enumerate(fengines):
                    eng.dma_start(
                        out=ft[:, e * half : (e + 1) * half, :],
                        in_=feat_ap[
                            :,
                            c * CHUNK_BLKS + e * half : c * CHUNK_BLKS
                            + (e + 1) * half,
                            :,
                        ],
                    )
                gt = gpool.tile([P, CHUNK_BLKS, FEAT], cdt)
                gh = CHUNK_BLKS // GSPLIT
                for g in range(GSPLIT):
                    sidx = nidx // GSPLIT
                    nc.gpsimd.dma_gather(
                        out_ap=gt[:, g * gh : (g + 1) * gh, :],
                        in_ap=cent_d.ap(),
                        idxs_ap=idx_t[
                            :,
                            c * (nidx // 16) + g * (sidx // 16) : c * (nidx // 16)
                            + (g + 1) * (sidx // 16),
                        ],
                        num_idxs=sidx,
                        num_idxs_reg=sidx,
                        elem_size=FEAT,
                        queue_num=((c * GSPLIT + g) % GQ_SPREAD)
                        if GQ_SPREAD
                        else 0,
                    )
                d_chunk = small.tile([P, CHUNK_BLKS], f32)
                for j in range(CHUNK_BLKS):
                    diff = dpool.tile([P, FEAT], ddt)
                    nc.vector.tensor_tensor(
                        out=diff[:],
                        in0=ft[:, j, :],
                        in1=gt[:, j, :],
                        op=mybir.AluOpType.subtract,
                    )
                    sq = sqpool.tile([P, FEAT], ddt)
                    if j < ACT_DIFF:
                        nc.scalar.activation(
                            out=sq[:],
                            in_=diff[:],
                            func=mybir.ActivationFunctionType.Square,
                            accum_out=d_chunk[:, j : j + 1],
                        )
                    else:
                        nc.vector.scalar_tensor_tensor(
                            out=sq[:],
                            in0=diff[:],
                            scalar=0.0,
                            in1=diff[:],
                            op0=mybir.AluOpType.bypass,
                            op1=mybir.AluOpType.mult,
                            accum_out=d_chunk[:, j : j + 1],
                        )
                if BATCH_ONEHOT:
                    mh = CHUNK_BLKS // MSPLIT
                    for m in range(MSPLIT):
                        ms = slice(
                            c * CHUNK_BLKS + m * mh,
                            c * CHUNK_BLKS + (m + 1) * mh,
                        )
                        nc.vector.tensor_tensor(
                            out=rhs_all[:, ms, :],
                            in0=ohr_all[:, ms, :],
                            in1=_bcast_ap(
                                d_chunk[:, m * mh : (m + 1) * mh],
                                [("d", mh), ("b", QW)],
                            ),
                            op=mybir.AluOpType.mult,
                        )
                for j in range(CHUNK_BLKS):
                    b = c * CHUNK_BLKS + j
                    if BATCH_ONEHOT:
                        lhsT = ohq_all[:, b, :]
                        rhs = rhs_all[:, b, :]
                    else:
                        ohq_tile = small.tile([P, QW], sdt, tag=f"oq{j % 4}")
                        nc.vector.tensor_scalar(
                            out=ohq_tile[:],
                            in0=iota_t[:],
                            scalar1=q_t[:, b : b + 1],
                            scalar2=None,
                            op0=mybir.AluOpType.is_equal,
                        )
                        rhs_tile = small.tile([P, QW], sdt, tag=f"rh{j % 4}")
                        nc.vector.tensor_scalar(
                            out=rhs_tile[:],
                            in0=iota_t[:],
                            scalar1=r_t[:, b : b + 1],
                            scalar2=d_chunk[:, j : j + 1],
                            op0=mybir.AluOpType.is_equal,
                            op1=mybir.AluOpType.mult,
                        )
                        lhsT = ohq_tile[:]
                        rhs = rhs_tile[:]
                    nc.tensor.matmul(
                        out=psum_t[:],
                        lhsT=lhsT,
                        rhs=rhs,
                        start=(b == 0),
                        stop=(b == NBLK - 1),
                    )
            res_t = singles.tile([QW, QW], f32)
            nc.vector.tensor_copy(out=res_t[:], in_=psum_t[:])
            nc.sync.dma_start(out=out_d.ap(), in_=res_t[:])

            if repeat > 1:
                loop_cm.__exit__(None, None, None)

    nc.compile()
    return nc


# ---------------------------------------------------------------------------
# Host-side prep / reduce

_LAST_PREP = None  # host metadata shared between make_in_maps and reduce


def _prep(features, centers, labels):
    """Choose the algorithm for this data and build all host metadata."""
    labels = np.asarray(labels).astype(np.int64, copy=False)
    counts = np.bincount(labels, minlength=NCLASS)[:NCLASS]
    prep = {"labels": labels, "counts": counts}
    if ALGO == "seg":
        order = np.argsort(labels, kind="stable")
        slab = labels[order]
        cls_per_core = []
        ok = True
        for c in range(NCORES):
            u = np.unique(slab[c * SHARD : (c + 1) * SHARD])
            if len(u) > KLOC:
                ok = False
            cls_per_core.append(u)
        if ok:
            prep.update(
                algo="seg", order=order, slab=slab, cls_per_core=cls_per_core
            )
            return prep
    prep["algo"] = "diff"
    return prep


def make_in_maps(features, centers, labels):
    """Host-side shard + layout prep. Returns list of 8 per-core input maps."""
    global _LAST_PREP
    features = np.ascontiguousarray(np.asarray(features), dtype=np.float32)
    centers = np.ascontiguousarray(np.asarray(centers), dtype=np.float32)
    prep = _prep(features, centers, labels)
    _LAST_PREP = prep
    labels = prep["labels"]

    if prep["algo"] == "seg":
        fdt = _np_dt(FEAT_DT)
        odt = _np_dt(OH_DT)
        counts = prep["counts"]
        w = np.zeros(NCLASS, np.float32)
        nz = counts > 0
        w[nz] = 1.0 / counts[nz]
        scaled = features * np.sqrt(w)[labels][:, None]
        fq = scaled.astype(fdt) if fdt is not np.float32 else scaled
        fs = fq[prep["order"]]  # sorted by label
        in_maps = []
        for c in range(NCORES):
            sl = prep["slab"][c * SHARD : (c + 1) * SHARD]
            u = prep["cls_per_core"][c]
            lcode = np.searchsorted(u, sl)  # [SHARD] in [0, len(u))
            oh = np.zeros((SHARD, KLOC), dtype=odt)
            oh[np.arange(SHARD), lcode] = 1.0
            # sample i = b*128 + p  ->  onehot[p, b, :]
            oh = np.ascontiguousarray(
                oh.reshape(NBLK, P, KLOC).transpose(1, 0, 2)
            )
            in_maps.append(
                {
                    "features": np.ascontiguousarray(
                        fs[c * SHARD : (c + 1) * SHARD]
                    ),
                    "onehot": oh,
                }
            )
        return in_maps

    # diff fallback
    fdt = _np_dt("bf16")
    cdt = _np_dt(CENT_DT)
    feats = features.astype(fdt) if fdt is not np.float32 else features
    cents = centers.astype(cdt) if cdt is not np.float32 else centers
    iota = np.ascontiguousarray(
        np.broadcast_to(np.arange(QW, dtype=np.float32), (P, QW))
    )
    in_maps = []
    for c in range(NCORES):
        lab = labels[c * SHARD : (c + 1) * SHARD]
        idx16 = np.ascontiguousarray(lab.reshape(SHARD // 16, 16).T).astype(
            np.int16
        )
        idx16 = np.ascontiguousarray(np.tile(idx16, (8, 1)))
        lab_blk = lab.reshape(NBLK, P).T  # [p, b] = lab[b*128+p]
        in_maps.append(
            {
                "features": feats[c * SHARD : (c + 1) * SHARD],
                "centers": cents,
                "labels16": idx16,
                "qcol": np.ascontiguousarray((lab_blk // QW).astype(np.float32)),
                "rcol": np.ascontiguousarray((lab_blk % QW).astype(np.float32)),
                "iota": iota,
            }
        )
    return in_maps


def reduce_outputs(res_list, centers):
    """Combine per-core device partials + host-side terms into the loss."""
    prep = _LAST_PREP
    counts = prep["counts"]
    cent64 = np.asarray(centers, dtype=np.float64)

    if prep["algo"] == "seg":
        sqrt_w = np.zeros(NCLASS)
        nz = counts > 0
        sqrt_w[nz] = np.sqrt(1.0 / counts[nz])
        a_total = 0.0  # sum_i ||g_i||^2 = sum_j w_j A_j
        t_total = 0.0  # sum_j sqrt(w_j) <S'_j, c_j> = sum_j w_j <S_j, c_j>
        for c in range(NCORES):
            u = prep["cls_per_core"][c]
            a_total += float(
                np.asarray(res_list[c]["s2"], dtype=np.float64).sum()
            )
            S = np.asarray(res_list[c]["S"], dtype=np.float64)[: len(u)]
            t_total += float(
                np.einsum("kf,kf->k", S, cent64[u]).dot(sqrt_w[u])
            )
        c2 = float(np.einsum("jf,jf->", cent64[nz], cent64[nz]))
        total = (a_total - 2.0 * t_total + c2) / (FEAT * BATCH)
        return np.asarray(total, dtype=np.float32)

    # diff fallback
    tot = np.sum(
        np.asarray([r["out"] for r in res_list], dtype=np.float64), axis=0
    )
    sums = tot.reshape(-1)[:NCLASS]
    per_class = np.where(counts > 0, sums / np.maximum(counts * FEAT, 1.0), 0.0)
    return np.asarray(per_class.sum() / BATCH, dtype=np.float32)


_MODULES = {}


def _get_module(algo):
    if algo not in _MODULES:
        _MODULES[algo] = build_module(algo=algo)
    return _MODULES[algo]


LAST_RESULT = None


def kernel(features, centers, labels):
    global LAST_RESULT
    in_maps = make_in_maps(features, centers, labels)
    nc = _get_module(_LAST_PREP["algo"])
    res = run_bass_kernel_spmd(
        nc, in_maps, core_ids=list(range(NCORES)), trace=TRACE
    )
    LAST_RESULT = res
    return reduce_outputs(res.results, np.asarray(centers, dtype=np.float32))
